# revision 1
# baseline (speedup 1.0000x reference)
# Neural-collapse regularizer (tr_SW / tr_SB) on 8 TRN2 NeuronCores.
#
# Math: with per-class sums S_c = sum_{i: l_i=c} x_i, counts n_c,
# ssq = sum_i ||x_i||^2:
#   tr_SW = ssq - sum_c ||S_c||^2 / n_c
#   tr_SB = sum_c ||S_c/n_c - g||^2,  g = (sum_c S_c) / N
# So the device only needs the segment sums [C, D] and ssq; everything
# else is tiny O(C*D) host math.
#
# Sharding: class-parallel. Core k owns classes [128k, 128(k+1)); the host
# routes each row to the core that owns its label (segment sum is
# order-invariant so any within-core row order is fine).
#
# Layout trick: rows are packed in chunks of GRP=8 rows of a single class,
# one chunk per (group, partition) slot. All 8 row-tiles of a group then
# share one [128x128] one-hot (built once per group on DVE) and one
# stationary operand for all the group's matmuls.
#
# ssq trick: each row's sum-of-squares s[p] is produced by a fused
# square+row-reduce (DVE scalar_tensor_tensor or ACT Square+accum_out,
# split across both engines for balance) directly into a bf16 "s slot"
# at column 512 of the row's 514-wide lane. The second matmul of each
# tile covers columns [256, 513): its 257th output column accumulates
# per-class sum-of-squares in PSUM for free.

import contextlib
import ctypes
import os
import sys
import types

import numpy as np
import ml_dtypes

import concourse.bass as bass
import concourse.bacc as bacc
import concourse.mybir as mybir
import concourse.tile as tile
from concourse.bass_utils import run_bass_kernel_spmd


def _ensure_ntff_hook():
    """Provide antenv.axon_hooks + an NTFF profile hook when the image's
    antenv package lacks it (needed only for trace=True timing runs)."""
    try:
        from antenv.axon_hooks import get_axon_ntff_profile_hook  # noqa: F401
        return
    except ImportError:
        pass
    mod = types.ModuleType("antenv.axon_hooks")
    state = {"hook": None}
    mod.set_axon_ntff_profile_hook = lambda h: state.__setitem__("hook", h)
    mod.get_axon_ntff_profile_hook = lambda: state["hook"]
    sys.modules["antenv.axon_hooks"] = mod

    so_path = "/opt/axon/libaxon_pjrt.so"
    if not os.path.exists(so_path):
        return
    lib = ctypes.CDLL(so_path)
    if not hasattr(lib, "axon_start_nrt_profile"):
        return
    lib.axon_start_nrt_profile.argtypes = [
        ctypes.POINTER(ctypes.c_int64), ctypes.c_size_t]
    lib.axon_start_nrt_profile.restype = ctypes.c_int64
    lib.axon_stop_nrt_profile.argtypes = [ctypes.c_char_p]
    lib.axon_stop_nrt_profile.restype = ctypes.c_int64

    @contextlib.contextmanager
    def _hook(output_dir, device_ids):
        import jax
        jax.devices()
        if device_ids:
            ids = (ctypes.c_int64 * len(device_ids))(*device_ids)
            rc = lib.axon_start_nrt_profile(ids, len(device_ids))
        else:
            rc = lib.axon_start_nrt_profile(None, 0)
        if rc != 0:
            raise RuntimeError(f"axon_start_nrt_profile rc={rc}")
        try:
            yield
        finally:
            n = lib.axon_stop_nrt_profile(str(output_dir).encode())
            print(f"profile: {n} file(s) written to {output_dir}",
                  file=sys.stderr)

    mod.set_axon_ntff_profile_hook(_hook)


CORES = 8
P = 128              # partitions = classes per core
D = 512              # feature dim (asserted against input)
GRP = 8              # row-tiles per group = rows per chunk
LANE = D + 2         # per-tile lane: 512 features, 1 s-slot, 1 pad (align)
HALF = D // 2
BF16 = mybir.dt.bfloat16
F32 = mybir.dt.float32
NP_BF16 = ml_dtypes.bfloat16

# Per-tile engine split for the sum-of-squares work (weights, any scale):
#   A: DVE scalar_tensor_tensor (square + row-reduce, 1x, ~604ns)
#   B: ACT Square + accum_out (~805ns)
#   C: DVE tensor_tensor mult (2x, ~327ns) + PE ones-matmul reduce (~240ns)
W_A = float(os.environ.get("K_W_A", "37"))
W_B = float(os.environ.get("K_W_B", "55"))
W_C = float(os.environ.get("K_W_C", "44"))


def _host_shard(features: np.ndarray, labels: np.ndarray):
    """Chunked class-sorted layout.

    Returns (in_maps, G). in_maps[k]:
      feat: [G, 128, GRP*LANE] bf16 -- slot (g, p) holds GRP rows of one
            class at j*LANE offsets; cols 512/513 of each lane are zero.
      lab:  [128, G] f32 -- rebased class (0..127) of slot (g, p)
      iota: [128, 128] bf16
    """
    N, d = features.shape
    assert d == D, f"expected D={D}, got {d}"
    CPAD = CORES * P

    order = np.argsort(labels, kind="stable")
    sl = labels[order]
    class_start = np.searchsorted(sl, np.arange(CPAD + 1))  # [1025]
    counts = np.diff(class_start)                            # [1024]
    chunks_per_class = -(-counts // GRP)                     # ceil
    core_chunks = chunks_per_class.reshape(CORES, P)
    G = int(-(-core_chunks.sum(axis=1).max() // P))

    fbf = features.astype(NP_BF16)
    iota = np.broadcast_to(np.arange(P, dtype=NP_BF16), (P, P)).copy()

    in_maps = []
    for k in range(CORES):
        nch = core_chunks[k]                    # chunks per rebased class
        total = int(nch.sum())
        assert total <= G * P
        # chunk m -> class: repeat
        chunk_cls = np.repeat(np.arange(P), nch)             # [total]
        # padded row grid: [G*P, GRP] of global row indices, -1 = empty
        grid = np.full((G * P, GRP), -1, dtype=np.int64)
        # scatter each class's rows into its chunks
        cls_pad_start = np.concatenate(([0], np.cumsum(nch * GRP)))  # [129]
        cnts = counts[k * P:(k + 1) * P]
        lo = class_start[k * P]
        n_k = int(cnts.sum())
        rows_k = order[lo:lo + n_k]
        lab_k = sl[lo:lo + n_k] - k * P          # rebased, sorted 0..127
        within = np.arange(n_k) - np.repeat(class_start[k * P:(k + 1) * P] - lo,
                                            cnts)
        pos = np.repeat(cls_pad_start[:-1], cnts) + within
        grid.reshape(-1)[pos] = rows_k

        # gather features; zero the padding rows
        safe = np.maximum(grid, 0)
        fr = fbf[safe.reshape(-1)]               # [G*P*GRP, D]
        fr[grid.reshape(-1) < 0] = 0
        fr = fr.reshape(G * P, GRP, D)

        feat = np.zeros((G * P, GRP, LANE), dtype=NP_BF16)
        feat[:, :, :D] = fr
        # chunk m -> (g = m // P, p = m % P)
        feat = feat.reshape(G, P, GRP * LANE)

        labg = np.zeros((G * P,), dtype=np.float32)
        labg[:total] = chunk_cls
        labg = np.ascontiguousarray(labg.reshape(G, P).T)    # [128, G]

        in_maps.append({"feat": feat, "lab": labg, "iota": iota})
    return in_maps, G



# per-group tile pattern: first N_A tiles -> DVE STT (fused square+row-sum),
# next N_B -> ACT Square+accum_out, last N_C -> DVE 2x TT-mult + PE
# ones-matmul reduce. Balances DVE/ACT/PE under the ~49us DMA floor.
N_A = int(os.environ.get("K_N_A", "2"))
N_B = int(os.environ.get("K_N_B", "3"))
N_C = GRP - N_A - N_B

XB = int(os.environ.get("K_XB", "8"))    # xg group buffers
OHB = 4                                   # one-hot buffers
SQCB = 3                                  # scratch rotation depth (groups)

def _build_raw(G: int):
    T = G * GRP
    nc = bacc.Bacc("TRN2", target_bir_lowering=False, debug=False,
                   enable_asserts=False)
    feat_h = nc.dram_tensor("feat", [G, P, GRP * LANE], BF16,
                            kind="ExternalInput")
    lab_h = nc.dram_tensor("lab", [P, G], F32, kind="ExternalInput")
    iota_h = nc.dram_tensor("iota", [P, P], BF16, kind="ExternalInput")
    out_h = nc.dram_tensor("out", [P, D + 2], F32, kind="ExternalOutput")

    x_sb = nc.alloc_sbuf_tensor("x_sb", [P, XB * GRP * LANE], BF16)
    oh_sb = nc.alloc_sbuf_tensor("oh_sb", [P, OHB * P], BF16)
    sqd_sb = nc.alloc_sbuf_tensor("sqd_sb", [P, SQCB * N_A * D], BF16)
    sqa_sb = nc.alloc_sbuf_tensor("sqa_sb", [P, SQCB * N_B * D], BF16)
    sqc_sb = nc.alloc_sbuf_tensor("sqc_sb", [P, SQCB * N_C * D], BF16)
    iota_sb = nc.alloc_sbuf_tensor("iota_sb", [P, P], BF16)
    lab_sb = nc.alloc_sbuf_tensor("lab_sb", [P, G], F32)
    ones_sb = nc.alloc_sbuf_tensor("ones_sb", [P, 1], BF16)
    out_sb = nc.alloc_sbuf_tensor("out_sb", [P, D + 2], F32)
    psum_a = nc.alloc_psum_tensor("psum_a", [P, D], F32)
    psum_b = nc.alloc_psum_tensor("psum_b", [P, D], F32)
    psum_c = nc.alloc_psum_tensor("psum_c", [P, D], F32)

    xg_ap = lambda g: x_sb.ap()[:, (g % XB) * GRP * LANE:
                                (g % XB + 1) * GRP * LANE]
    oh_ap = lambda g: oh_sb.ap()[:, (g % OHB) * P:(g % OHB + 1) * P]

    import contextlib as _ctx
    with (
        _ctx.ExitStack() as _sems,
                nc.semaphore("sem_oh") as sem_oh,
        nc.semaphore("sem_sd") as sem_sd,
        nc.semaphore("sem_sa") as sem_sa,
        nc.semaphore("sem_pe") as sem_pe,
        nc.semaphore("sem_cp") as sem_cp,
        nc.semaphore("sem_out") as sem_out,
        nc.semaphore("sem_ones") as sem_ones,
        nc.semaphore("sem_iota") as sem_iota,
        nc.semaphore("sem_lab") as sem_lab,
        nc.Block() as block,
    ):
        sem_xs = [_sems.enter_context(nc.semaphore(f"sem_x{b}"))
                  for b in range(XB)]

        def wait_x(eng, g):
            eng.wait_ge(sem_xs[g % XB], 16 * (g // XB + 1))
        @block.gpsimd
        def _(gpsimd):
            gpsimd.memset(ones_sb.ap(), 1.0)
            gpsimd.memset(out_sb.ap()[:, D + 1:D + 2], 0.0).then_inc(
                sem_ones, 1)


        @block.sync
        def _(sync):
            sync.dma_start(out=xg_ap(0), in_=feat_h.ap()[0]).then_inc(
                sem_xs[0], 16)
            sync.dma_start(out=iota_sb.ap(), in_=iota_h.ap()).then_inc(
                sem_iota, 16)
            sync.dma_start(out=lab_sb.ap(), in_=lab_h.ap()).then_inc(
                sem_lab, 16)
            for g in range(1, G):
                if g >= XB:
                    sync.wait_ge(sem_pe, g - XB + 1)
                sync.dma_start(out=xg_ap(g), in_=feat_h.ap()[g]).then_inc(
                    sem_xs[g % XB], 16)
            sync.wait_ge(sem_cp, 1)
            sync.dma_start(out=out_h.ap(), in_=out_sb.ap()).then_inc(
                sem_out, 16)
            sync.wait_ge(sem_out, 16)

        @block.vector
        def _(vector):
            vector.wait_ge(sem_iota, 16)
            vector.wait_ge(sem_lab, 16)
            with nc.allow_low_precision("bf16 row sums; aggregate err ~1e-5"):
                for g in range(G):
                    wait_x(vector, g)
                    if g >= SQCB:
                        vector.wait_ge(sem_pe, g - SQCB + 1)
                    xg = xg_ap(g)
                    vector.tensor_scalar(
                        oh_ap(g), iota_sb.ap(), lab_sb.ap()[:, g:g + 1], None,
                        mybir.AluOpType.is_equal,
                    ).then_inc(sem_oh, 1)
                    last = None
                    for j in range(N_A):
                        off = j * LANE
                        dbuf = (g % SQCB) * N_A + j
                        last = vector.scalar_tensor_tensor(
                            out=sqd_sb.ap()[:, dbuf * D:(dbuf + 1) * D],
                            in0=xg[:, off:off + D], scalar=1.0,
                            in1=xg[:, off:off + D],
                            op0=mybir.AluOpType.mult,
                            op1=mybir.AluOpType.mult,
                            accum_out=xg[:, off + D:off + D + 1],
                        )
                    for i in range(N_C):
                        j = N_A + N_B + i
                        off = j * LANE
                        buf = (g % SQCB) * N_C + i
                        last = vector.tensor_tensor(
                            out=sqc_sb.ap()[:, buf * D:(buf + 1) * D],
                            in0=xg[:, off:off + D], in1=xg[:, off:off + D],
                            op=mybir.AluOpType.mult,
                        )
                    assert last is not None
                    last.then_inc(sem_sd, 1)
                # tail: copy psums out
                vector.wait_ge(sem_pe, G)
                vector.wait_ge(sem_ones, 1)
                vector.tensor_copy(out=out_sb.ap()[:, 0:HALF],
                                   in_=psum_a.ap()[:, 0:HALF])
                vector.tensor_copy(out=out_sb.ap()[:, HALF:D + 1],
                                   in_=psum_b.ap()[:, 0:HALF + 1])
                vector.tensor_reduce(
                    out=out_sb.ap()[0:1, D + 1:D + 2],
                    in_=psum_c.ap()[0:1, :],
                    axis=mybir.AxisListType.X, op=mybir.AluOpType.add,
                ).then_inc(sem_cp, 1)

        @block.scalar
        def _(scalar):
            with nc.allow_low_precision("bf16 row sums; aggregate err ~1e-5"):
                for g in range(G):
                    wait_x(scalar, g)
                    if g >= SQCB:
                        scalar.wait_ge(sem_pe, g - SQCB + 1)
                    xg = xg_ap(g)
                    last = None
                    for i in range(N_B):
                        j = N_A + i
                        off = j * LANE
                        abuf = (g % SQCB) * N_B + i
                        last = scalar.activation(
                            sqa_sb.ap()[:, abuf * D:(abuf + 1) * D],
                            xg[:, off:off + D],
                            mybir.ActivationFunctionType.Square,
                            accum_out=xg[:, off + D:off + D + 1],
                        )
                    last.then_inc(sem_sa, 1)

        @block.tensor
        def _(tensor):
            tensor.wait_ge(sem_ones, 1)
            for g in range(G):
                t0 = g * GRP
                tensor.wait_ge(sem_oh, g + 1)
                wait_x(tensor, g)
                xg = xg_ap(g)
                oh = oh_ap(g)
                for j in range(GRP):
                    off = j * LANE
                    tensor.matmul(
                        out=psum_a.ap()[:, 0:HALF], lhsT=oh,
                        rhs=xg[:, off:off + HALF],
                        start=(t0 + j == 0), stop=(t0 + j == T - 1),
                    )
                tensor.wait_ge(sem_sd, g + 1)
                tensor.wait_ge(sem_sa, g + 1)
                last = None
                for j in range(GRP):
                    off = j * LANE
                    last = tensor.matmul(
                        out=psum_b.ap()[:, 0:HALF + 1], lhsT=oh,
                        rhs=xg[:, off + HALF:off + D + 1],
                        start=(t0 + j == 0), stop=(t0 + j == T - 1),
                    )
                for i in range(N_C):
                    buf = (g % SQCB) * N_C + i
                    last = tensor.matmul(
                        out=psum_c.ap()[0:1, :], lhsT=ones_sb.ap(),
                        rhs=sqc_sb.ap()[:, buf * D:(buf + 1) * D],
                        start=(g == 0 and i == 0),
                        stop=(g == G - 1 and i == N_C - 1),
                    )
                last.then_inc(sem_pe, 1)

    nc.compile()
    return nc



def _finalize(results, labels: np.ndarray, C: int, N: int):
    sums = np.concatenate(
        [np.asarray(r["out"][:, :D], dtype=np.float64) for r in results], axis=0
    )  # [1024, D]
    ssq = float(sum(np.asarray(r["out"][:, D], dtype=np.float64).sum()
                    + float(r["out"][0, D + 1])
                    for r in results))
    counts = np.bincount(labels, minlength=CORES * P).astype(np.float64)

    sums = sums[:C]
    counts = counts[:C]
    means = sums / counts[:, None]
    g = sums.sum(axis=0) / N
    tr_sw = ssq - float(((sums * sums).sum(axis=1) / counts).sum())
    tr_sb = float(((means - g) ** 2).sum())
    return np.asarray(np.float32(tr_sw / tr_sb))


def run(features, labels, num_classes, trace=False):
    features = np.asarray(features, dtype=np.float32)
    labels = np.asarray(labels).astype(np.int64).ravel()
    C = int(num_classes)
    N = features.shape[0]
    assert C <= CORES * P, f"num_classes={C} exceeds {CORES * P}"

    if trace:
        _ensure_ntff_hook()
    in_maps, G = _host_shard(features, labels)
    nc = _build_raw(G)
    res = run_bass_kernel_spmd(nc, in_maps, list(range(CORES)), trace=trace)
    out = _finalize(res.results, labels, C, N)
    return out, res


def kernel(**inputs) -> np.ndarray:
    trace = os.environ.get("KERNEL_TRACE", "0") == "1"
    out, _ = run(inputs["features"], inputs["labels"], inputs["num_classes"],
                 trace=trace)
    return out



# revision 12
# speedup vs baseline: 1.2928x; 1.2928x over previous
# Neural-collapse regularizer (tr_SW / tr_SB) on 8 TRN2 NeuronCores.
#
# Math: with per-class sums S_c = sum_{i: l_i=c} x_i, counts n_c,
# ssq = sum_i ||x_i||^2:
#   tr_SW = ssq - sum_c ||S_c||^2 / n_c
#   tr_SB = sum_c ||S_c/n_c - g||^2,  g = (sum_c S_c) / N
# So the device only needs the segment sums [C, D] and ssq; everything
# else is tiny O(C*D) host math.
#
# Sharding: class-parallel. Core k owns classes [128k, 128(k+1)); the host
# routes each row to the core that owns its label (segment sum is
# order-invariant so any within-core row order is fine).
#
# Layout: rows are packed in chunks of GRP=8 rows of a single class,
# one chunk per (group, partition) slot; features are shipped as fp8e4
# (halves HBM traffic vs bf16; quantization error ~0.1% on the final
# ratio, far under the 2e-2 gate). Tiles j and j+4 of a group form a
# "pair" [128, 2, 512] that feeds fp8 DoubleRow matmuls (2 k-tiles per
# pass, 0.5 cycles/col).
#
# Per group (8 tiles = 4 pairs):
#   PE    : pairs {0,1} -> Gram diagonal trick: accumulate
#           X_b^T X_b for the four 128-col blocks into ONE [128,128]
#           psum across all groups; its diagonal sums to the ssq of
#           those pairs. Plus 4 DoubleRow class-sum matmuls (all pairs)
#           sharing the group's one-hot, plus 2 plain bf16 ones-matmuls
#           reducing the DVE pair's squares.
#   DVE   : pair 2 squares (tensor_tensor fp8->bf16 scratch).
#   ACT   : pair 3 squares (Square activation, fused accum_out -> per-
#           group per-partition partial; the wide output is a dump).
#   GPSIMD: builds the one-hot [128, 2, 128] fp8 for the group.
# ssq = gram-diag + ones-psum + act-accums, all reduced in a tiny tail.

import contextlib
import ctypes
import os
import sys
import types

import numpy as np
import ml_dtypes

import concourse.bass as bass
import concourse.bacc as bacc
import concourse.mybir as mybir
from concourse.bass_utils import run_bass_kernel_spmd


def _ensure_ntff_hook():
    """Provide antenv.axon_hooks + an NTFF profile hook when the image's
    antenv package lacks it (needed only for trace=True timing runs)."""
    try:
        from antenv.axon_hooks import get_axon_ntff_profile_hook  # noqa: F401
        return
    except ImportError:
        pass
    mod = types.ModuleType("antenv.axon_hooks")
    state = {"hook": None}
    mod.set_axon_ntff_profile_hook = lambda h: state.__setitem__("hook", h)
    mod.get_axon_ntff_profile_hook = lambda: state["hook"]
    sys.modules["antenv.axon_hooks"] = mod

    so_path = "/opt/axon/libaxon_pjrt.so"
    if not os.path.exists(so_path):
        return
    lib = ctypes.CDLL(so_path)
    if not hasattr(lib, "axon_start_nrt_profile"):
        return
    lib.axon_start_nrt_profile.argtypes = [
        ctypes.POINTER(ctypes.c_int64), ctypes.c_size_t]
    lib.axon_start_nrt_profile.restype = ctypes.c_int64
    lib.axon_stop_nrt_profile.argtypes = [ctypes.c_char_p]
    lib.axon_stop_nrt_profile.restype = ctypes.c_int64

    @contextlib.contextmanager
    def _hook(output_dir, device_ids):
        import jax
        jax.devices()
        if device_ids:
            ids = (ctypes.c_int64 * len(device_ids))(*device_ids)
            rc = lib.axon_start_nrt_profile(ids, len(device_ids))
        else:
            rc = lib.axon_start_nrt_profile(None, 0)
        if rc != 0:
            raise RuntimeError(f"axon_start_nrt_profile rc={rc}")
        try:
            yield
        finally:
            n = lib.axon_stop_nrt_profile(str(output_dir).encode())
            print(f"profile: {n} file(s) written to {output_dir}",
                  file=sys.stderr)

    mod.set_axon_ntff_profile_hook(_hook)


CORES = 8
P = 128              # partitions = classes per core
D = 512              # feature dim (asserted against input)
GRP = 8              # row-tiles per group = rows per chunk
HALF = D // 2
BF16 = mybir.dt.bfloat16
F32 = mybir.dt.float32
FP8 = mybir.dt.float8e4
NP_BF16 = ml_dtypes.bfloat16
NP_FP8 = ml_dtypes.float8_e4m3

# out columns: 512 class sums, 512=gram diag, 513=act accum, 514=ones
# partial (row 0 only), 515 pad
OUTW = D + 4

XB = int(os.environ.get("K_XB", "16"))   # x group buffers
OHB = 4                                  # one-hot buffers
SQR = 3                                  # sq scratch rotation depth
OH_ON_GPS = os.environ.get("K_OH", "V") == "G"
NO_GRAM = os.environ.get("K_NO_GRAM", "0") == "1"
NO_CLS = os.environ.get("K_NO_CLS", "0") == "1"
NO_ONES = os.environ.get("K_NO_ONES", "0") == "1"
NO_TAIL = os.environ.get("K_NO_TAIL", "0") == "1"


def _host_shard(features: np.ndarray, labels: np.ndarray):
    """Chunked class-sorted layout.

    Returns (in_maps, G). in_maps[k]:
      feat:  [G, 128, GRP*D] fp8e4 -- slot (g, p) holds GRP rows of one
             class at j*D offsets.
      lab:   [128, G] f32 -- rebased class (0..127) of slot (g, p)
      iota2: [128, 2*128] bf16 -- two copies of 0..127 per partition
      id128: [128, 128] bf16 -- identity matrix
    """
    N, d = features.shape
    assert d == D, f"expected D={D}, got {d}"
    CPAD = CORES * P

    order = np.argsort(labels, kind="stable")
    sl = labels[order]
    class_start = np.searchsorted(sl, np.arange(CPAD + 1))  # [1025]
    counts = np.diff(class_start)                            # [1024]
    chunks_per_class = -(-counts // GRP)                     # ceil
    core_chunks = chunks_per_class.reshape(CORES, P)
    G = int(-(-core_chunks.sum(axis=1).max() // P))

    f8 = features.astype(NP_FP8)
    iota = np.arange(P, dtype=NP_BF16)
    iota2 = np.ascontiguousarray(
        np.broadcast_to(np.concatenate([iota, iota]), (P, 2 * P)))
    id128 = np.eye(P, dtype=np.float32)

    in_maps = []
    for k in range(CORES):
        nch = core_chunks[k]                    # chunks per rebased class
        total = int(nch.sum())
        assert total <= G * P
        chunk_cls = np.repeat(np.arange(P), nch)             # [total]
        # padded row grid: [G*P, GRP] of global row indices, -1 = empty
        grid = np.full((G * P, GRP), -1, dtype=np.int64)
        cls_pad_start = np.concatenate(([0], np.cumsum(nch * GRP)))  # [129]
        cnts = counts[k * P:(k + 1) * P]
        lo = class_start[k * P]
        n_k = int(cnts.sum())
        rows_k = order[lo:lo + n_k]
        within = np.arange(n_k) - np.repeat(class_start[k * P:(k + 1) * P] - lo,
                                            cnts)
        pos = np.repeat(cls_pad_start[:-1], cnts) + within
        grid.reshape(-1)[pos] = rows_k

        safe = np.maximum(grid, 0)
        fr = f8[safe.reshape(-1)]               # [G*P*GRP, D]
        fr[grid.reshape(-1) < 0] = 0
        feat = np.ascontiguousarray(
            fr.reshape(G, P, GRP * D))

        labg = np.zeros((G * P,), dtype=np.float32)
        labg[:total] = chunk_cls
        labg = np.ascontiguousarray(labg.reshape(G, P).T)    # [128, G]

        in_maps.append({"feat": feat, "lab": labg, "iota2": iota2,
                        "id128": id128})
    return in_maps, G


def _build_raw(G: int):
    nc = bacc.Bacc("TRN2", target_bir_lowering=False, debug=False,
                   enable_asserts=False)
    feat_h = nc.dram_tensor("feat", [G, P, GRP * D], FP8,
                            kind="ExternalInput")
    lab_h = nc.dram_tensor("lab", [P, G], F32, kind="ExternalInput")
    iota2_h = nc.dram_tensor("iota2", [P, 2 * P], BF16, kind="ExternalInput")
    id_h = nc.dram_tensor("id128", [P, P], F32, kind="ExternalInput")
    out_h = nc.dram_tensor("out", [P, OUTW], F32, kind="ExternalOutput")

    x_sb = nc.alloc_sbuf_tensor("x_sb", [P, XB, GRP, D], FP8)
    oh_sb = nc.alloc_sbuf_tensor("oh_sb", [P, OHB, 2, P], FP8)
    sq_sb = nc.alloc_sbuf_tensor("sq_sb", [P, SQR, 2, D], BF16)
    dump_sb = nc.alloc_sbuf_tensor("dump_sb", [P, 2, 2, D], BF16)
    acc_sb = nc.alloc_sbuf_tensor("acc_sb", [P, G], F32)
    gd_sb = nc.alloc_sbuf_tensor("gd_sb", [P, P], F32)
    gd2_sb = nc.alloc_sbuf_tensor("gd2_sb", [P, P], F32)
    iota2_sb = nc.alloc_sbuf_tensor("iota2_sb", [P, 2, P], BF16)
    id_sb = nc.alloc_sbuf_tensor("id_sb", [P, P], F32)
    lab_sb = nc.alloc_sbuf_tensor("lab_sb", [P, G], F32)
    ones_sb = nc.alloc_sbuf_tensor("ones_sb", [P, 1], BF16)
    out_sb = nc.alloc_sbuf_tensor("out_sb", [P, OUTW], F32)
    psum_cls = nc.alloc_psum_tensor("psum_cls", [P, D], F32)
    psum_gram = nc.alloc_psum_tensor("psum_gram", [P, P], F32)
    psum_ones = nc.alloc_psum_tensor("psum_ones", [P, D], F32)

    DR = mybir.MatmulPerfMode.DoubleRow

    import contextlib as _ctx
    with (
        _ctx.ExitStack() as _sems,
        nc.semaphore("sem_oh") as sem_oh,
        nc.semaphore("sem_sqd") as sem_sqd,
        nc.semaphore("sem_sqa") as sem_sqa,
        nc.semaphore("sem_pe") as sem_pe,
        nc.semaphore("sem_cp") as sem_cp,
        nc.semaphore("sem_out") as sem_out,
        nc.semaphore("sem_ones") as sem_ones,
        nc.semaphore("sem_iota") as sem_iota,
        nc.semaphore("sem_id") as sem_id,
        nc.semaphore("sem_lab") as sem_lab,
        nc.Block() as block,
    ):
        sem_xs = [_sems.enter_context(nc.semaphore(f"sem_x{b}"))
                  for b in range(XB)]

        def wait_x(eng, g):
            eng.wait_ge(sem_xs[g % XB], 16 * (g // XB + 1))

        # pair t of group g: tiles (t, t+4) -> AP [128, 2, 512]
        def xpair(g, t):
            return x_sb.ap()[:, g % XB, t:t + 5:4, :]

        def xpair_blk(g, t, b):
            return x_sb.ap()[:, g % XB, t:t + 5:4, b * P:(b + 1) * P]

        @block.gpsimd
        def _(gpsimd):
            gpsimd.memset(ones_sb.ap(), 1.0)
            gpsimd.memset(out_sb.ap()[:, D + 2:D + 4], 0.0).then_inc(
                sem_ones, 1)
            if OH_ON_GPS:
                gpsimd.wait_ge(sem_iota, 16)
                gpsimd.wait_ge(sem_lab, 16)
                for g in range(G):
                    if g >= OHB:
                        gpsimd.wait_ge(sem_pe, g - OHB + 1)
                    gpsimd.tensor_scalar(
                        oh_sb.ap()[:, g % OHB], iota2_sb.ap(),
                        lab_sb.ap()[:, g:g + 1], None,
                        mybir.AluOpType.is_equal,
                    ).then_inc(sem_oh, 1)

        @block.sync
        def _(sync):
            sync.dma_start(out=x_sb.ap()[:, 0], in_=feat_h.ap()[0]).then_inc(
                sem_xs[0], 16)
            sync.dma_start(out=iota2_sb.ap(), in_=iota2_h.ap()).then_inc(
                sem_iota, 16)
            sync.dma_start(out=lab_sb.ap(), in_=lab_h.ap()).then_inc(
                sem_lab, 16)
            sync.dma_start(out=id_sb.ap(), in_=id_h.ap()).then_inc(
                sem_id, 16)
            for g in range(1, G):
                if g >= XB:
                    sync.wait_ge(sem_pe, g - XB + 1)
                    sync.wait_ge(sem_sqa, g - XB + 1)
                sync.dma_start(out=x_sb.ap()[:, g % XB],
                               in_=feat_h.ap()[g]).then_inc(
                    sem_xs[g % XB], 16)
            sync.wait_ge(sem_cp, 2)
            sync.dma_start(out=out_h.ap(), in_=out_sb.ap()).then_inc(
                sem_out, 16)
            sync.wait_ge(sem_out, 16)

        @block.vector
        def _(vector):
            if not OH_ON_GPS:
                vector.wait_ge(sem_iota, 16)
                vector.wait_ge(sem_lab, 16)
            with nc.allow_low_precision("bf16 squares; aggregate err ~1e-4"):
                for g in range(G):
                    wait_x(vector, g)
                    if g >= SQR:
                        vector.wait_ge(sem_pe, g - SQR + 1)
                    if not OH_ON_GPS:
                        if g >= OHB:
                            vector.wait_ge(sem_pe, g - OHB + 1)
                        vector.tensor_scalar(
                            oh_sb.ap()[:, g % OHB], iota2_sb.ap(),
                            lab_sb.ap()[:, g:g + 1], None,
                            mybir.AluOpType.is_equal,
                        ).then_inc(sem_oh, 1)
                    xp = xpair(g, 2)
                    vector.tensor_tensor(
                        out=sq_sb.ap()[:, g % SQR], in0=xp, in1=xp,
                        op=mybir.AluOpType.mult,
                    ).then_inc(sem_sqd, 1)
                # tail: gram diag, psum copies, reductions
                vector.wait_ge(sem_pe, G)
                vector.wait_ge(sem_sqa, G)
                vector.wait_ge(sem_id, 16)
                vector.wait_ge(sem_ones, 1)
                if NO_TAIL:
                    vector.memset(out_sb.ap()[:, 0:HALF], 0.0)
                    vector.memset(out_sb.ap()[:, D:D + 3], 0.0).then_inc(
                        sem_cp, 1)
                else:
                    vector.tensor_copy(out=gd_sb.ap(), in_=psum_gram.ap())
                    vector.scalar_tensor_tensor(
                        out=gd2_sb.ap(), in0=gd_sb.ap(), scalar=1.0,
                        in1=id_sb.ap(), op0=mybir.AluOpType.mult,
                        op1=mybir.AluOpType.mult,
                        accum_out=out_sb.ap()[:, D:D + 1],
                    )
                    vector.tensor_copy(out=out_sb.ap()[:, 0:HALF],
                                       in_=psum_cls.ap()[:, 0:HALF])
                    vector.tensor_copy(out=out_sb.ap()[:, HALF:D],
                                       in_=psum_cls.ap()[:, HALF:D])
                    vector.tensor_reduce(
                        out=out_sb.ap()[:, D + 1:D + 2], in_=acc_sb.ap(),
                        axis=mybir.AxisListType.X, op=mybir.AluOpType.add,
                    )
                    vector.tensor_reduce(
                        out=out_sb.ap()[0:1, D + 2:D + 3],
                        in_=psum_ones.ap()[0:1, :],
                        axis=mybir.AxisListType.X, op=mybir.AluOpType.add,
                    ).then_inc(sem_cp, 1)

        @block.scalar
        def _(scalar):
            with nc.allow_low_precision("bf16 squares; aggregate err ~1e-4"):
                for g in range(G):
                    wait_x(scalar, g)
                    if g >= 2:
                        # WAW on the rotating dump slot: wait own retire
                        scalar.wait_ge(sem_sqa, g - 1)
                    scalar.activation(
                        dump_sb.ap()[:, g % 2], xpair(g, 3),
                        mybir.ActivationFunctionType.Square,
                        accum_out=acc_sb.ap()[:, g:g + 1],
                    ).then_inc(sem_sqa, 1)
                # tail handled entirely on vector
                scalar.sem_inc(sem_cp, 1)

        @block.tensor
        def _(tensor):
            tensor.wait_ge(sem_ones, 1)
            for g in range(G):
                wait_x(tensor, g)
                # gram pairs 0,1: 4 diag-block DR matmuls each
                if not NO_GRAM:
                    for t in range(2):
                        for b in range(4):
                            tensor.matmul(
                                out=psum_gram.ap(),
                                lhsT=xpair_blk(g, t, b),
                                rhs=xpair_blk(g, t, b),
                                start=(g == 0 and t == 0 and b == 0),
                                stop=(g == G - 1 and t == 1 and b == 3),
                                perf_mode=DR,
                            )
                # class sums: 4 DR matmuls sharing oh2
                tensor.wait_ge(sem_oh, g + 1)
                if not NO_CLS:
                    for t in range(4):
                        tensor.matmul(
                            out=psum_cls.ap(), lhsT=oh_sb.ap()[:, g % OHB],
                            rhs=xpair(g, t),
                            start=(g == 0 and t == 0),
                            stop=(g == G - 1 and t == 3),
                            perf_mode=DR,
                        )
                # ones-reduce of DVE pair squares (plain bf16 matmuls)
                tensor.wait_ge(sem_sqd, g + 1)
                if not NO_ONES:
                    last = None
                    for i in range(2):
                        last = tensor.matmul(
                            out=psum_ones.ap()[0:1, :], lhsT=ones_sb.ap(),
                            rhs=sq_sb.ap()[:, g % SQR, i, :],
                            start=(g == 0 and i == 0),
                            stop=(g == G - 1 and i == 1),
                        )
                    last.then_inc(sem_pe, 1)
                else:
                    tensor.sem_inc(sem_pe, 1)

    nc.compile()
    return nc


def _finalize(results, labels: np.ndarray, C: int, N: int):
    sums = np.concatenate(
        [np.asarray(r["out"][:, :D], dtype=np.float64) for r in results], axis=0
    )  # [1024, D]
    ssq = float(sum(np.asarray(r["out"][:, D], dtype=np.float64).sum()
                    + np.asarray(r["out"][:, D + 1], dtype=np.float64).sum()
                    + float(r["out"][0, D + 2])
                    for r in results))
    counts = np.bincount(labels, minlength=CORES * P).astype(np.float64)

    sums = sums[:C]
    counts = counts[:C]
    means = sums / counts[:, None]
    g = sums.sum(axis=0) / N
    tr_sw = ssq - float(((sums * sums).sum(axis=1) / counts).sum())
    tr_sb = float(((means - g) ** 2).sum())
    return np.asarray(np.float32(tr_sw / tr_sb))


def run(features, labels, num_classes, trace=False):
    features = np.asarray(features, dtype=np.float32)
    labels = np.asarray(labels).astype(np.int64).ravel()
    C = int(num_classes)
    N = features.shape[0]
    assert C <= CORES * P, f"num_classes={C} exceeds {CORES * P}"

    if trace:
        _ensure_ntff_hook()
    in_maps, G = _host_shard(features, labels)
    nc = _build_raw(G)
    res = run_bass_kernel_spmd(nc, in_maps, list(range(CORES)), trace=trace)
    out = _finalize(res.results, labels, C, N)
    return out, res


def kernel(**inputs) -> np.ndarray:
    trace = os.environ.get("KERNEL_TRACE", "0") == "1"
    out, _ = run(inputs["features"], inputs["labels"], inputs["num_classes"],
                 trace=trace)
    return out


# revision 18
# speedup vs baseline: 1.4055x; 1.0872x over previous
# Neural-collapse regularizer (tr_SW / tr_SB) on 8 TRN2 NeuronCores.
#
# Math: with per-class sums S_c = sum_{i: l_i=c} x_i, counts n_c,
# ssq = sum_i ||x_i||^2:
#   tr_SW = ssq - sum_c ||S_c||^2 / n_c
#   tr_SB = sum_c ||S_c/n_c - g||^2,  g = (sum_c S_c) / N
# The device computes the segment sums [C, D] and ssq; everything else
# is tiny O(C*D) host math.
#
# Sharding: class-parallel. Core k owns classes [128k, 128(k+1)); the
# host routes each row to the core that owns its label.
#
# Layout: rows are packed in chunks of GRP=8 rows of one class, one
# chunk per (group, partition) slot; features ship as fp8e4 (halves
# HBM traffic vs bf16; ~2e-3 error on the final ratio vs the 2e-2
# gate). Each group's 4352B partition line = 8 x-tiles (8*512) + the
# slot's one-hot duplicated twice (2*128), so the one-hot costs no
# compute and no extra DMA round. Tiles j and j+4 form a "pair"
# [128, 2, 512] feeding fp8 DoubleRow matmuls (2 k-tiles per pass).
#
# Per group (8 tiles = 4 pairs):
#   PE  : pairs {0,1} -> Gram-diagonal ssq (accumulate X_b^T X_b for
#         the four 128-col blocks into ONE [128,128] psum across all
#         groups; its diagonal sums to those pairs' ssq), plus 4
#         DoubleRow class-sum matmuls sharing the shipped one-hot.
#   DVE : pair 2 -> scalar_tensor_tensor square with fused accum_out
#         (per-group per-partition partials; wide output is a dump).
#   ACT : pair 3 -> Square activation with fused accum_out.
# ssq = gram diag + DVE accums + ACT accums, reduced in a short tail.

import contextlib
import ctypes
import os
import sys
import types

import numpy as np
import ml_dtypes

import concourse.bass as bass
import concourse.bacc as bacc
import concourse.mybir as mybir
from concourse.bass_utils import run_bass_kernel_spmd


def _ensure_ntff_hook():
    """Provide antenv.axon_hooks + an NTFF profile hook when the image's
    antenv package lacks it (needed only for trace=True timing runs)."""
    try:
        from antenv.axon_hooks import get_axon_ntff_profile_hook  # noqa: F401
        return
    except ImportError:
        pass
    mod = types.ModuleType("antenv.axon_hooks")
    state = {"hook": None}
    mod.set_axon_ntff_profile_hook = lambda h: state.__setitem__("hook", h)
    mod.get_axon_ntff_profile_hook = lambda: state["hook"]
    sys.modules["antenv.axon_hooks"] = mod

    so_path = "/opt/axon/libaxon_pjrt.so"
    if not os.path.exists(so_path):
        return
    lib = ctypes.CDLL(so_path)
    if not hasattr(lib, "axon_start_nrt_profile"):
        return
    lib.axon_start_nrt_profile.argtypes = [
        ctypes.POINTER(ctypes.c_int64), ctypes.c_size_t]
    lib.axon_start_nrt_profile.restype = ctypes.c_int64
    lib.axon_stop_nrt_profile.argtypes = [ctypes.c_char_p]
    lib.axon_stop_nrt_profile.restype = ctypes.c_int64

    @contextlib.contextmanager
    def _hook(output_dir, device_ids):
        import jax
        jax.devices()
        if device_ids:
            ids = (ctypes.c_int64 * len(device_ids))(*device_ids)
            rc = lib.axon_start_nrt_profile(ids, len(device_ids))
        else:
            rc = lib.axon_start_nrt_profile(None, 0)
        if rc != 0:
            raise RuntimeError(f"axon_start_nrt_profile rc={rc}")
        try:
            yield
        finally:
            n = lib.axon_stop_nrt_profile(str(output_dir).encode())
            print(f"profile: {n} file(s) written to {output_dir}",
                  file=sys.stderr)

    mod.set_axon_ntff_profile_hook(_hook)


CORES = 8
P = 128              # partitions = classes per core
D = 512              # feature dim (asserted against input)
GRP = 8              # row-tiles per group = rows per chunk
HALF = D // 2
LINE = GRP * D + 2 * P   # 4352: 8 x-tiles + doubled one-hot
BF16 = mybir.dt.bfloat16
F32 = mybir.dt.float32
FP8 = mybir.dt.float8e4
NP_BF16 = ml_dtypes.bfloat16
NP_FP8 = ml_dtypes.float8_e4m3

# out columns: 512 class sums, 512=gram diag, 513=ACT accums, 514=DVE
# accums
OUTW = D + 3

XB = int(os.environ.get("K_XB", "16"))        # x line buffers
NWARM = int(os.environ.get("K_NWARM", "24"))  # PE clock-ramp dummies


def _host_shard(features: np.ndarray, labels: np.ndarray):
    """Chunked class-sorted layout.

    Returns (in_maps, G). in_maps[k]:
      feat:  [G, 128, LINE] fp8e4 -- slot (g, p): GRP rows of one class
             at j*D offsets, then its one-hot (0..127) duplicated 2x.
      id128: [128, 128] f32 -- identity matrix (gram diag extraction)
    """
    N, d = features.shape
    assert d == D, f"expected D={D}, got {d}"
    CPAD = CORES * P

    order = np.argsort(labels, kind="stable")
    sl = labels[order]
    class_start = np.searchsorted(sl, np.arange(CPAD + 1))  # [1025]
    counts = np.diff(class_start)                            # [1024]
    chunks_per_class = -(-counts // GRP)                     # ceil
    core_chunks = chunks_per_class.reshape(CORES, P)
    G = int(-(-core_chunks.sum(axis=1).max() // P))

    f8 = features.astype(NP_FP8)
    eye = np.eye(P, dtype=NP_FP8)
    eye2 = np.concatenate([eye, eye], axis=1)                # [128, 256]
    id128 = np.eye(P, dtype=np.float32)

    in_maps = []
    for k in range(CORES):
        nch = core_chunks[k]                    # chunks per rebased class
        total = int(nch.sum())
        assert total <= G * P
        chunk_cls = np.repeat(np.arange(P), nch)             # [total]
        # padded row grid: [G*P, GRP] of global row indices, -1 = empty
        grid = np.full((G * P, GRP), -1, dtype=np.int64)
        cls_pad_start = np.concatenate(([0], np.cumsum(nch * GRP)))  # [129]
        cnts = counts[k * P:(k + 1) * P]
        lo = class_start[k * P]
        n_k = int(cnts.sum())
        rows_k = order[lo:lo + n_k]
        within = np.arange(n_k) - np.repeat(class_start[k * P:(k + 1) * P] - lo,
                                            cnts)
        pos = np.repeat(cls_pad_start[:-1], cnts) + within
        grid.reshape(-1)[pos] = rows_k

        safe = np.maximum(grid, 0)
        fr = f8[safe.reshape(-1)]               # [G*P*GRP, D]
        fr[grid.reshape(-1) < 0] = 0

        slot_cls = np.zeros((G * P,), dtype=np.int64)
        slot_cls[:total] = chunk_cls

        feat = np.empty((G * P, LINE), dtype=NP_FP8)
        feat[:, :GRP * D] = fr.reshape(G * P, GRP * D)
        feat[:, GRP * D:] = eye2[slot_cls]
        feat = np.ascontiguousarray(feat.reshape(G, P, LINE))

        in_maps.append({"feat": feat, "id128": id128})
    return in_maps, G


def _build_raw(G: int):
    nc = bacc.Bacc("TRN2", target_bir_lowering=False, debug=False,
                   enable_asserts=False)
    feat_h = nc.dram_tensor("feat", [G, P, LINE], FP8, kind="ExternalInput")
    id_h = nc.dram_tensor("id128", [P, P], F32, kind="ExternalInput")
    out_h = nc.dram_tensor("out", [P, OUTW], F32, kind="ExternalOutput")

    x_sb = nc.alloc_sbuf_tensor("x_sb", [P, XB, LINE], FP8)
    dumpd_sb = nc.alloc_sbuf_tensor("dumpd_sb", [P, 2, 2, D], BF16)
    dumpa_sb = nc.alloc_sbuf_tensor("dumpa_sb", [P, 2, 2, D], BF16)
    acca_sb = nc.alloc_sbuf_tensor("acca_sb", [P, G], F32)
    accd_sb = nc.alloc_sbuf_tensor("accd_sb", [P, G], F32)
    gd_sb = nc.alloc_sbuf_tensor("gd_sb", [P, P], F32)
    gd2_sb = nc.alloc_sbuf_tensor("gd2_sb", [P, P], F32)
    id_sb = nc.alloc_sbuf_tensor("id_sb", [P, P], F32)
    warm_sb = nc.alloc_sbuf_tensor("warm_sb", [P, 64], BF16)
    out_sb = nc.alloc_sbuf_tensor("out_sb", [P, OUTW], F32)
    psum_cls = nc.alloc_psum_tensor("psum_cls", [P, D], F32)
    psum_gram = nc.alloc_psum_tensor("psum_gram", [P, P], F32)
    psum_warm = nc.alloc_psum_tensor("psum_warm", [P, 64], F32)

    DR = mybir.MatmulPerfMode.DoubleRow

    import contextlib as _ctx
    with (
        _ctx.ExitStack() as _sems,
        nc.semaphore("sem_sqd") as sem_sqd,
        nc.semaphore("sem_sqa") as sem_sqa,
        nc.semaphore("sem_pe") as sem_pe,
        nc.semaphore("sem_cp") as sem_cp,
        nc.semaphore("sem_out") as sem_out,
        nc.semaphore("sem_warm") as sem_warm,
        nc.semaphore("sem_gd") as sem_gd,
        nc.semaphore("sem_id") as sem_id,
        nc.Block() as block,
    ):
        sem_xs = [_sems.enter_context(nc.semaphore(f"sem_x{b}"))
                  for b in range(XB)]

        def wait_x(eng, g):
            eng.wait_ge(sem_xs[g % XB], 16 * (g // XB + 1))

        # pair t of group g: tiles (t, t+4) -> AP [128, 2, 512]
        def xpair(g, t):
            sl = x_sb.ap()[:, g % XB, t * D:(t + 5) * D]
            return sl.rearrange("p (a b) -> p a b", a=5)[:, 0:5:4, :]

        def xpair_blk(g, t, b):
            sl = x_sb.ap()[:, g % XB, t * D:(t + 5) * D]
            return sl.rearrange("p (a b) -> p a b", a=5)[:, 0:5:4,
                                                         b * P:(b + 1) * P]

        def ohpair(g):
            sl = x_sb.ap()[:, g % XB, GRP * D:]
            return sl.rearrange("p (a b) -> p a b", a=2)

        @block.gpsimd
        def _(gpsimd):
            gpsimd.memset(warm_sb.ap(), 0.0).then_inc(sem_warm, 1)

        @block.sync
        def _(sync):
            sync.dma_start(out=x_sb.ap()[:, 0], in_=feat_h.ap()[0]).then_inc(
                sem_xs[0], 16)
            sync.dma_start(out=id_sb.ap(), in_=id_h.ap()).then_inc(
                sem_id, 16)
            for g in range(1, G):
                if g >= XB:
                    sync.wait_ge(sem_pe, g - XB + 1)
                    sync.wait_ge(sem_sqd, g - XB + 1)
                    sync.wait_ge(sem_sqa, g - XB + 1)
                sync.dma_start(out=x_sb.ap()[:, g % XB],
                               in_=feat_h.ap()[g]).then_inc(
                    sem_xs[g % XB], 16)
            sync.wait_ge(sem_cp, 1)
            sync.dma_start(out=out_h.ap(), in_=out_sb.ap()).then_inc(
                sem_out, 16)
            sync.wait_ge(sem_out, 16)

        @block.vector
        def _(vector):
            with nc.allow_low_precision("bf16 dump; accums are f32"):
                for g in range(G):
                    wait_x(vector, g)
                    if g >= 2:
                        vector.wait_ge(sem_sqd, g - 1)  # dump slot WAW
                    xp = xpair(g, 2)
                    vector.scalar_tensor_tensor(
                        out=dumpd_sb.ap()[:, g % 2], in0=xp, scalar=1.0,
                        in1=xp, op0=mybir.AluOpType.mult,
                        op1=mybir.AluOpType.mult,
                        accum_out=accd_sb.ap()[:, g:g + 1],
                    ).then_inc(sem_sqd, 1)
                # tail: gram diag, psum copies, accum reductions
                vector.wait_ge(sem_pe, G)
                vector.wait_ge(sem_sqa, G)
                vector.wait_ge(sem_sqd, G)
                vector.wait_ge(sem_id, 16)
                vector.tensor_copy(out=gd_sb.ap(),
                                   in_=psum_gram.ap()).then_inc(sem_gd, 1)
                vector.wait_ge(sem_gd, 1)
                vector.scalar_tensor_tensor(
                    out=gd2_sb.ap(), in0=gd_sb.ap(), scalar=1.0,
                    in1=id_sb.ap(), op0=mybir.AluOpType.mult,
                    op1=mybir.AluOpType.mult,
                    accum_out=out_sb.ap()[:, D:D + 1],
                )
                vector.tensor_copy(out=out_sb.ap()[:, 0:HALF],
                                   in_=psum_cls.ap()[:, 0:HALF])
                vector.tensor_copy(out=out_sb.ap()[:, HALF:D],
                                   in_=psum_cls.ap()[:, HALF:D])
                vector.tensor_reduce(
                    out=out_sb.ap()[:, D + 1:D + 2], in_=acca_sb.ap(),
                    axis=mybir.AxisListType.X, op=mybir.AluOpType.add,
                )
                vector.tensor_reduce(
                    out=out_sb.ap()[:, D + 2:D + 3], in_=accd_sb.ap(),
                    axis=mybir.AxisListType.X, op=mybir.AluOpType.add,
                ).then_inc(sem_cp, 1)

        @block.scalar
        def _(scalar):
            with nc.allow_low_precision("bf16 dump; accums are f32"):
                for g in range(G):
                    wait_x(scalar, g)
                    if g >= 2:
                        scalar.wait_ge(sem_sqa, g - 1)  # dump slot WAW
                    scalar.activation(
                        dumpa_sb.ap()[:, g % 2], xpair(g, 3),
                        mybir.ActivationFunctionType.Square,
                        accum_out=acca_sb.ap()[:, g:g + 1],
                    ).then_inc(sem_sqa, 1)

        @block.tensor
        def _(tensor):
            # clock-ramp warmup: garbage accumulation chain, never read
            tensor.wait_ge(sem_warm, 1)
            for w in range(NWARM):
                tensor.matmul(
                    out=psum_warm.ap()[0:64, :], lhsT=warm_sb.ap()[:, 0:64],
                    rhs=warm_sb.ap(),
                    start=(w == 0), stop=(w == NWARM - 1),
                )
            for g in range(G):
                wait_x(tensor, g)
                # gram pairs 0,1: 4 diag-block DR matmuls each
                for t in range(2):
                    for b in range(4):
                        tensor.matmul(
                            out=psum_gram.ap(),
                            lhsT=xpair_blk(g, t, b),
                            rhs=xpair_blk(g, t, b),
                            start=(g == 0 and t == 0 and b == 0),
                            stop=(g == G - 1 and t == 1 and b == 3),
                            perf_mode=DR,
                        )
                # class sums: 4 DR matmuls sharing the shipped one-hot
                last = None
                for t in range(4):
                    last = tensor.matmul(
                        out=psum_cls.ap(), lhsT=ohpair(g),
                        rhs=xpair(g, t),
                        start=(g == 0 and t == 0),
                        stop=(g == G - 1 and t == 3),
                        perf_mode=DR,
                    )
                last.then_inc(sem_pe, 1)

    nc.compile()
    return nc


def _finalize(results, labels: np.ndarray, C: int, N: int):
    sums = np.concatenate(
        [np.asarray(r["out"][:, :D], dtype=np.float64) for r in results], axis=0
    )  # [1024, D]
    ssq = float(sum(np.asarray(r["out"][:, D:], dtype=np.float64).sum()
                    for r in results))
    counts = np.bincount(labels, minlength=CORES * P).astype(np.float64)

    sums = sums[:C]
    counts = counts[:C]
    means = sums / counts[:, None]
    g = sums.sum(axis=0) / N
    tr_sw = ssq - float(((sums * sums).sum(axis=1) / counts).sum())
    tr_sb = float(((means - g) ** 2).sum())
    return np.asarray(np.float32(tr_sw / tr_sb))


def run(features, labels, num_classes, trace=False):
    features = np.asarray(features, dtype=np.float32)
    labels = np.asarray(labels).astype(np.int64).ravel()
    C = int(num_classes)
    N = features.shape[0]
    assert C <= CORES * P, f"num_classes={C} exceeds {CORES * P}"

    if trace:
        _ensure_ntff_hook()
    in_maps, G = _host_shard(features, labels)
    nc = _build_raw(G)
    res = run_bass_kernel_spmd(nc, in_maps, list(range(CORES)), trace=trace)
    out = _finalize(res.results, labels, C, N)
    return out, res


def kernel(**inputs) -> np.ndarray:
    trace = os.environ.get("KERNEL_TRACE", "0") == "1"
    out, _ = run(inputs["features"], inputs["labels"], inputs["num_classes"],
                 trace=trace)
    return out


# revision 21
# speedup vs baseline: 1.5466x; 1.1004x over previous
# Neural-collapse regularizer (tr_SW / tr_SB) on 8 TRN2 NeuronCores.
#
# Math: with per-class sums S_c = sum_{i: l_i=c} x_i, counts n_c,
# ssq = sum_i ||x_i||^2:
#   tr_SW = ssq - sum_c ||S_c||^2 / n_c
#   tr_SB = sum_c ||S_c/n_c - g||^2,  g = (sum_c S_c) / N
# The device computes the segment sums [C, D] and ssq; everything else
# is tiny O(C*D) host math.
#
# Sharding: class-parallel. Core k owns classes [128k, 128(k+1)); the
# host routes each row to the core that owns its label.
#
# Layout: rows are packed in chunks of GRP=8 rows of one class, one
# chunk per (group, partition) slot; features ship as fp8e4 (halves
# HBM traffic vs bf16; ~2e-3 error on the final ratio vs the 2e-2
# gate). Each group's 4352B partition line = 8 x-tiles (8*512) + the
# slot's one-hot duplicated twice (2*128), so the one-hot costs no
# compute and no extra DMA round. Tiles j and j+4 form a "pair"
# [128, 2, 512] feeding fp8 DoubleRow matmuls (2 k-tiles per pass).
#
# Per group (8 tiles = 4 pairs):
#   PE  : pairs {0,1} -> Gram-diagonal ssq (accumulate X_b^T X_b for
#         the four 128-col blocks into ONE [128,128] psum across all
#         groups; its diagonal sums to those pairs' ssq), plus 4
#         DoubleRow class-sum matmuls sharing the shipped one-hot.
#   DVE : pair 2 -> scalar_tensor_tensor square with fused accum_out
#         (per-group per-partition partials; wide output is a dump).
#   ACT : pair 3 -> Square activation with fused accum_out.
# ssq = gram diag + DVE accums + ACT accums, reduced in a short tail.

import contextlib
import ctypes
import os
import sys
import types

import numpy as np
import ml_dtypes

import concourse.bass as bass
import concourse.bacc as bacc
import concourse.mybir as mybir
from concourse.bass_utils import run_bass_kernel_spmd


def _ensure_ntff_hook():
    """Provide antenv.axon_hooks + an NTFF profile hook when the image's
    antenv package lacks it (needed only for trace=True timing runs)."""
    try:
        from antenv.axon_hooks import get_axon_ntff_profile_hook  # noqa: F401
        return
    except ImportError:
        pass
    mod = types.ModuleType("antenv.axon_hooks")
    state = {"hook": None}
    mod.set_axon_ntff_profile_hook = lambda h: state.__setitem__("hook", h)
    mod.get_axon_ntff_profile_hook = lambda: state["hook"]
    sys.modules["antenv.axon_hooks"] = mod

    so_path = "/opt/axon/libaxon_pjrt.so"
    if not os.path.exists(so_path):
        return
    lib = ctypes.CDLL(so_path)
    if not hasattr(lib, "axon_start_nrt_profile"):
        return
    lib.axon_start_nrt_profile.argtypes = [
        ctypes.POINTER(ctypes.c_int64), ctypes.c_size_t]
    lib.axon_start_nrt_profile.restype = ctypes.c_int64
    lib.axon_stop_nrt_profile.argtypes = [ctypes.c_char_p]
    lib.axon_stop_nrt_profile.restype = ctypes.c_int64

    @contextlib.contextmanager
    def _hook(output_dir, device_ids):
        import jax
        jax.devices()
        if device_ids:
            ids = (ctypes.c_int64 * len(device_ids))(*device_ids)
            rc = lib.axon_start_nrt_profile(ids, len(device_ids))
        else:
            rc = lib.axon_start_nrt_profile(None, 0)
        if rc != 0:
            raise RuntimeError(f"axon_start_nrt_profile rc={rc}")
        try:
            yield
        finally:
            n = lib.axon_stop_nrt_profile(str(output_dir).encode())
            print(f"profile: {n} file(s) written to {output_dir}",
                  file=sys.stderr)

    mod.set_axon_ntff_profile_hook(_hook)


CORES = 8
P = 128              # partitions = classes per core
D = 512              # feature dim (asserted against input)
GRP = 8              # row-tiles per group = rows per chunk
HALF = D // 2
LINE = GRP * D + 2 * P   # 4352: 8 x-tiles + doubled one-hot
BF16 = mybir.dt.bfloat16
F32 = mybir.dt.float32
FP8 = mybir.dt.float8e4
NP_BF16 = ml_dtypes.bfloat16
NP_FP8 = ml_dtypes.float8_e4m3

# out columns: 512 class sums, 512=gram diag, 513=ACT accums, 514=DVE
# accums
OUTW = D + 3

XB = int(os.environ.get("K_XB", "8"))         # x buffers (2 groups each)
NWARM = int(os.environ.get("K_NWARM", "24"))  # PE clock-ramp dummies


def _host_shard(features: np.ndarray, labels: np.ndarray):
    """Chunked class-sorted layout.

    Returns (in_maps, G). in_maps[k]:
      feat:  [G, 128, LINE] fp8e4 -- slot (g, p): GRP rows of one class
             at j*D offsets, then its one-hot (0..127) duplicated 2x.
      id128: [128, 128] f32 -- identity matrix (gram diag extraction)
    """
    N, d = features.shape
    assert d == D, f"expected D={D}, got {d}"
    CPAD = CORES * P

    order = np.argsort(labels, kind="stable")
    sl = labels[order]
    class_start = np.searchsorted(sl, np.arange(CPAD + 1))  # [1025]
    counts = np.diff(class_start)                            # [1024]
    chunks_per_class = -(-counts // GRP)                     # ceil
    core_chunks = chunks_per_class.reshape(CORES, P)
    G = int(-(-core_chunks.sum(axis=1).max() // P))

    f8 = features.astype(NP_FP8)
    eye = np.eye(P, dtype=NP_FP8)
    eye2 = np.concatenate([eye, eye], axis=1)                # [128, 256]
    id128 = np.eye(P, dtype=np.float32)

    in_maps = []
    for k in range(CORES):
        nch = core_chunks[k]                    # chunks per rebased class
        total = int(nch.sum())
        assert total <= G * P
        chunk_cls = np.repeat(np.arange(P), nch)             # [total]
        # padded row grid: [G*P, GRP] of global row indices, -1 = empty
        grid = np.full((G * P, GRP), -1, dtype=np.int64)
        cls_pad_start = np.concatenate(([0], np.cumsum(nch * GRP)))  # [129]
        cnts = counts[k * P:(k + 1) * P]
        lo = class_start[k * P]
        n_k = int(cnts.sum())
        rows_k = order[lo:lo + n_k]
        within = np.arange(n_k) - np.repeat(class_start[k * P:(k + 1) * P] - lo,
                                            cnts)
        pos = np.repeat(cls_pad_start[:-1], cnts) + within
        grid.reshape(-1)[pos] = rows_k

        safe = np.maximum(grid, 0)
        fr = f8[safe.reshape(-1)]               # [G*P*GRP, D]
        fr[grid.reshape(-1) < 0] = 0

        slot_cls = np.zeros((G * P,), dtype=np.int64)
        slot_cls[:total] = chunk_cls

        feat = np.empty((G * P, LINE), dtype=NP_FP8)
        feat[:, :GRP * D] = fr.reshape(G * P, GRP * D)
        feat[:, GRP * D:] = eye2[slot_cls]
        feat = feat.reshape(G, P, LINE)
        # pack group pairs into one 2*LINE partition line (fewer, larger
        # DMA batches); odd G -> pad with a never-transferred dummy half
        G2 = (G + 1) // 2
        feat2 = np.zeros((G2, P, 2, LINE), dtype=NP_FP8)
        for g in range(G):
            feat2[g // 2, :, g % 2, :] = feat[g]
        feat2 = np.ascontiguousarray(feat2.reshape(G2, P, 2 * LINE))

        in_maps.append({"feat": feat2, "id128": id128})
    return in_maps, G


def _build_raw(G: int):
    nc = bacc.Bacc("TRN2", target_bir_lowering=False, debug=False,
                   enable_asserts=False)
    G2 = (G + 1) // 2
    feat_h = nc.dram_tensor("feat", [G2, P, 2 * LINE], FP8,
                            kind="ExternalInput")
    id_h = nc.dram_tensor("id128", [P, P], F32, kind="ExternalInput")
    out_h = nc.dram_tensor("out", [P, OUTW], F32, kind="ExternalOutput")

    x_sb = nc.alloc_sbuf_tensor("x_sb", [P, XB, 2, LINE], FP8)
    dumpd_sb = nc.alloc_sbuf_tensor("dumpd_sb", [P, 2, 2, D], BF16)
    dumpa_sb = nc.alloc_sbuf_tensor("dumpa_sb", [P, 2, 2, D], BF16)
    acca_sb = nc.alloc_sbuf_tensor("acca_sb", [P, G], F32)
    accd_sb = nc.alloc_sbuf_tensor("accd_sb", [P, G], F32)
    gd_sb = nc.alloc_sbuf_tensor("gd_sb", [P, P], F32)
    gd2_sb = nc.alloc_sbuf_tensor("gd2_sb", [P, P], F32)
    id_sb = nc.alloc_sbuf_tensor("id_sb", [P, P], F32)
    warm_sb = nc.alloc_sbuf_tensor("warm_sb", [P, 64], BF16)
    out_sb = nc.alloc_sbuf_tensor("out_sb", [P, OUTW], F32)
    psum_cls = nc.alloc_psum_tensor("psum_cls", [P, D], F32)
    psum_gram = nc.alloc_psum_tensor("psum_gram", [P, P], F32)
    psum_warm = nc.alloc_psum_tensor("psum_warm", [P, 64], F32)

    DR = mybir.MatmulPerfMode.DoubleRow

    import contextlib as _ctx
    with (
        _ctx.ExitStack() as _sems,
        nc.semaphore("sem_sqd") as sem_sqd,
        nc.semaphore("sem_sqa") as sem_sqa,
        nc.semaphore("sem_pe") as sem_pe,
        nc.semaphore("sem_cp") as sem_cp,
        nc.semaphore("sem_out") as sem_out,
        nc.semaphore("sem_warm") as sem_warm,
        nc.semaphore("sem_gd") as sem_gd,
        nc.semaphore("sem_id") as sem_id,
        nc.Block() as block,
    ):
        sem_xs = [_sems.enter_context(nc.semaphore(f"sem_x{b}"))
                  for b in range(XB)]

        def wait_x(eng, g):
            sg = g // 2
            eng.wait_ge(sem_xs[sg % XB], 16 * (sg // XB + 1))

        def xline(g):
            return x_sb.ap()[:, (g // 2) % XB, g % 2]

        # pair t of group g: tiles (t, t+4) -> AP [128, 2, 512]
        def xpair(g, t):
            sl = xline(g)[:, t * D:(t + 5) * D]
            return sl.rearrange("p (a b) -> p a b", a=5)[:, 0:5:4, :]

        def xpair_blk(g, t, b):
            sl = xline(g)[:, t * D:(t + 5) * D]
            return sl.rearrange("p (a b) -> p a b", a=5)[:, 0:5:4,
                                                         b * P:(b + 1) * P]

        def ohpair(g):
            sl = xline(g)[:, GRP * D:]
            return sl.rearrange("p (a b) -> p a b", a=2)

        @block.gpsimd
        def _(gpsimd):
            gpsimd.memset(warm_sb.ap(), 0.0).then_inc(sem_warm, 1)

        @block.sync
        def _(sync):
            for sg in range(G2):
                if sg >= XB:
                    done = 2 * (sg - XB) + 2   # groups of buffer sg-XB
                    sync.wait_ge(sem_pe, done)
                    sync.wait_ge(sem_sqd, done)
                    sync.wait_ge(sem_sqa, done)
                if 2 * sg + 1 >= G and G % 2 == 1:
                    # last (odd) group: transfer only the real half-line
                    sync.dma_start(
                        out=x_sb.ap()[:, sg % XB, 0],
                        in_=feat_h.ap()[sg][:, 0:LINE]).then_inc(
                        sem_xs[sg % XB], 16)
                else:
                    sync.dma_start(out=x_sb.ap()[:, sg % XB],
                                   in_=feat_h.ap()[sg]).then_inc(
                        sem_xs[sg % XB], 16)
                if sg == 0:
                    sync.dma_start(out=id_sb.ap(), in_=id_h.ap()).then_inc(
                        sem_id, 16)
            sync.wait_ge(sem_cp, 1)
            sync.dma_start(out=out_h.ap(), in_=out_sb.ap()).then_inc(
                sem_out, 16)
            sync.wait_ge(sem_out, 16)

        @block.vector
        def _(vector):
            with nc.allow_low_precision("bf16 dump; accums are f32"):
                for g in range(G):
                    wait_x(vector, g)
                    if g >= 2:
                        vector.wait_ge(sem_sqd, g - 1)  # dump slot WAW
                    xp = xpair(g, 2)
                    vector.scalar_tensor_tensor(
                        out=dumpd_sb.ap()[:, g % 2], in0=xp, scalar=1.0,
                        in1=xp, op0=mybir.AluOpType.mult,
                        op1=mybir.AluOpType.mult,
                        accum_out=accd_sb.ap()[:, g:g + 1],
                    ).then_inc(sem_sqd, 1)
                # tail: gram diag, psum copies, accum reductions
                vector.wait_ge(sem_pe, G)
                vector.wait_ge(sem_sqa, G)
                vector.wait_ge(sem_sqd, G)
                vector.wait_ge(sem_id, 16)
                vector.tensor_copy(out=gd_sb.ap(),
                                   in_=psum_gram.ap()).then_inc(sem_gd, 1)
                vector.wait_ge(sem_gd, 1)
                vector.scalar_tensor_tensor(
                    out=gd2_sb.ap(), in0=gd_sb.ap(), scalar=1.0,
                    in1=id_sb.ap(), op0=mybir.AluOpType.mult,
                    op1=mybir.AluOpType.mult,
                    accum_out=out_sb.ap()[:, D:D + 1],
                )
                vector.tensor_copy(out=out_sb.ap()[:, 0:HALF],
                                   in_=psum_cls.ap()[:, 0:HALF])
                vector.tensor_copy(out=out_sb.ap()[:, HALF:D],
                                   in_=psum_cls.ap()[:, HALF:D])
                vector.tensor_reduce(
                    out=out_sb.ap()[:, D + 1:D + 2], in_=acca_sb.ap(),
                    axis=mybir.AxisListType.X, op=mybir.AluOpType.add,
                )
                vector.tensor_reduce(
                    out=out_sb.ap()[:, D + 2:D + 3], in_=accd_sb.ap(),
                    axis=mybir.AxisListType.X, op=mybir.AluOpType.add,
                ).then_inc(sem_cp, 1)

        @block.scalar
        def _(scalar):
            with nc.allow_low_precision("bf16 dump; accums are f32"):
                for g in range(G):
                    wait_x(scalar, g)
                    if g >= 2:
                        scalar.wait_ge(sem_sqa, g - 1)  # dump slot WAW
                    scalar.activation(
                        dumpa_sb.ap()[:, g % 2], xpair(g, 3),
                        mybir.ActivationFunctionType.Square,
                        accum_out=acca_sb.ap()[:, g:g + 1],
                    ).then_inc(sem_sqa, 1)

        @block.tensor
        def _(tensor):
            # clock-ramp warmup: garbage accumulation chain, never read
            tensor.wait_ge(sem_warm, 1)
            for w in range(NWARM):
                tensor.matmul(
                    out=psum_warm.ap()[0:64, :], lhsT=warm_sb.ap()[:, 0:64],
                    rhs=warm_sb.ap(),
                    start=(w == 0), stop=(w == NWARM - 1),
                )
            for g in range(G):
                wait_x(tensor, g)
                # gram pairs 0,1: 4 diag-block DR matmuls each
                for t in range(2):
                    for b in range(4):
                        tensor.matmul(
                            out=psum_gram.ap(),
                            lhsT=xpair_blk(g, t, b),
                            rhs=xpair_blk(g, t, b),
                            start=(g == 0 and t == 0 and b == 0),
                            stop=(g == G - 1 and t == 1 and b == 3),
                            perf_mode=DR,
                        )
                # class sums: 4 DR matmuls sharing the shipped one-hot
                last = None
                for t in range(4):
                    last = tensor.matmul(
                        out=psum_cls.ap(), lhsT=ohpair(g),
                        rhs=xpair(g, t),
                        start=(g == 0 and t == 0),
                        stop=(g == G - 1 and t == 3),
                        perf_mode=DR,
                    )
                last.then_inc(sem_pe, 1)

    nc.compile()
    return nc


def _finalize(results, labels: np.ndarray, C: int, N: int):
    sums = np.concatenate(
        [np.asarray(r["out"][:, :D], dtype=np.float64) for r in results], axis=0
    )  # [1024, D]
    ssq = float(sum(np.asarray(r["out"][:, D:], dtype=np.float64).sum()
                    for r in results))
    counts = np.bincount(labels, minlength=CORES * P).astype(np.float64)

    sums = sums[:C]
    counts = counts[:C]
    means = sums / counts[:, None]
    g = sums.sum(axis=0) / N
    tr_sw = ssq - float(((sums * sums).sum(axis=1) / counts).sum())
    tr_sb = float(((means - g) ** 2).sum())
    return np.asarray(np.float32(tr_sw / tr_sb))


def run(features, labels, num_classes, trace=False):
    features = np.asarray(features, dtype=np.float32)
    labels = np.asarray(labels).astype(np.int64).ravel()
    C = int(num_classes)
    N = features.shape[0]
    assert C <= CORES * P, f"num_classes={C} exceeds {CORES * P}"

    if trace:
        _ensure_ntff_hook()
    in_maps, G = _host_shard(features, labels)
    nc = _build_raw(G)
    res = run_bass_kernel_spmd(nc, in_maps, list(range(CORES)), trace=trace)
    out = _finalize(res.results, labels, C, N)
    return out, res


def kernel(**inputs) -> np.ndarray:
    trace = os.environ.get("KERNEL_TRACE", "0") == "1"
    out, _ = run(inputs["features"], inputs["labels"], inputs["num_classes"],
                 trace=trace)
    return out
